# revision 33
# baseline (speedup 1.0000x reference)
"""DMPNN message-passing kernel for 8 Trainium2 NeuronCores.

Strategy (hardcoded for E=8192, N=4096, H=256, T=4):
  - Shard edges across the 8 cores (1024 rows of adj_ee each); shard nodes
    for the aggregation stage (512 rows of adj_ne each).
  - Ship adjacency shards pre-transposed and cast to fp8e4 (exact for a 0/1
    adjacency) -> quarters the dominant HBM traffic; the PE runs the mixed
    bf16(stationary) x fp8(moving) matmul at full bf16 rate (fp32 matmul is
    quarter-rate on trn2).
  - Per iteration each core computes m.T = h.T @ A_shard.T on the PE
    (stationary = h k-tiles, moving = adjacency), does LayerNorm with
    broadcast-stat matmul tricks (ones[128,128] @ m.T gives column sums
    replicated over partitions), folds the LN gain into host-precomputed
    W2 = diag(g) @ W_pass[t], applies the residual + relu, and AllGathers
    the updated bf16 h shard.
  - Final stage: node aggregation (adj_ne), LN, dense+relu, LN, per-core
    column sum; host sums the 8 per-core partials.

Numerics: bf16 is used only where it is exact (adjacency) or benign
(h values / LN stats; measured ~6e-4 rel err end-to-end); the m @ W_pass
matmul and everything after it stays fp32.
"""

import numpy as np
import ml_dtypes

E, N, NODE_D, EDGE_D, H, T = 8192, 4096, 64, 16, 256, 4
NCORES = 8
EC = E // NCORES       # 1024 edges per core
NNC = N // NCORES      # 512 nodes per core
KT = E // 128          # 64 contraction tiles
ES = EC // 128         # 8 edge subtiles per core
ET = EC // 512         # 2 edge 512-col tiles per core
NS = NNC // 128        # 4 node subtiles per core
EPS = 1e-6
BF = ml_dtypes.bfloat16
F8 = ml_dtypes.float8_e4m3

LAST_RESULT = None     # BassKernelResults of the most recent run (for test.py)

_prog_cache = {}


def _build(flags, repeat=1, nocc=False):
    import concourse.bacc as bacc
    import concourse.mybir as mybir
    import concourse.tile as tile

    f32 = mybir.dt.float32
    bf16 = mybir.dt.bfloat16
    AF = mybir.ActivationFunctionType
    rg = [list(range(NCORES))]

    nc = bacc.Bacc("TRN2", target_bir_lowering=False, debug=False,
                   num_devices=NCORES)

    f8 = mybir.dt.float8e4
    aT = nc.declare_dram_parameter("aT", [E, EC], f8, isOutput=False)
    aneT = nc.declare_dram_parameter("aneT", [E, NNC], f8, isOutput=False)
    XcT = nc.declare_dram_parameter("XcT", [128, EC], f32, isOutput=False)
    Wi = nc.declare_dram_parameter("Wi", [128, H], f32, isOutput=False)
    W2s = nc.declare_dram_parameter("W2s", [T, H, H], f32, isOutput=False)
    nfT = nc.declare_dram_parameter("nfT", [128, NNC], f32, isOutput=False)
    Wn = nc.declare_dram_parameter("Wn", [128, H], f32, isOutput=False)
    Wm = nc.declare_dram_parameter("Wm", [H, H], f32, isOutput=False)
    c1 = nc.declare_dram_parameter("c1", [1, H], f32, isOutput=False)
    c2s = nc.declare_dram_parameter("c2s", [T, H], f32, isOutput=False)
    c3 = nc.declare_dram_parameter("c3", [1, H], f32, isOutput=False)
    gagg = nc.declare_dram_parameter("gagg", [1, H], f32, isOutput=False)
    bagg = nc.declare_dram_parameter("bagg", [1, H], f32, isOutput=False)
    out = nc.declare_dram_parameter("out", [1, H], f32, isOutput=True)

    ag_in = [nc.dram_tensor(f"agin{t}", [EC, H], bf16) for t in range(T + 1)]
    ag_out = [nc.dram_tensor(f"agout{t}", [E, H], bf16, addr_space="Shared")
              for t in range(T + 1)]

    with tile.TileContext(nc) as tc:
        with (
            tc.tile_pool(name="singles", bufs=1) as singles,
            tc.tile_pool(name="a", bufs=12) as apool,
            tc.tile_pool(name="hb", bufs=1) as hbpool,
            tc.tile_pool(name="hsh", bufs=2) as hshpool,
            tc.tile_pool(name="work", bufs=2) as work,
            tc.tile_pool(name="ps", bufs=7, space="PSUM") as pspool,
            tc.tile_pool(name="psout", bufs=1, space="PSUM") as psoutpool,
        ):
            # ---- static tiles ----
            xct_sb = singles.tile([128, EC], f32)
            nc.sync.dma_start(xct_sb[:], XcT[:, :])
            wi_sb = singles.tile([128, H], f32)
            nc.sync.dma_start(wi_sb[:], Wi[:, :])
            w2_sb = singles.tile([128, T, 2, H], f32)
            nc.sync.dma_start(
                w2_sb[:], W2s.ap().rearrange("t (kk p) n -> p t kk n", p=128))
            nft_sb = singles.tile([128, NNC], f32)
            nc.sync.dma_start(nft_sb[:], nfT[:, :])
            wn_sb = singles.tile([128, H], f32)
            nc.sync.dma_start(wn_sb[:], Wn[:, :])
            wm_sb = singles.tile([128, 2, H], f32)
            nc.sync.dma_start(
                wm_sb[:], Wm.ap().rearrange("(kk p) n -> p kk n", p=128))
            ones_bf = singles.tile([128, 128], bf16)
            nc.vector.memset(ones_bf[:], 1.0)
            onescol = singles.tile([128, 1], f32)
            nc.vector.memset(onescol[:], 1.0)
            eps_sb = singles.tile([128, 1], f32)
            nc.vector.memset(eps_sb[:], EPS)
            h0_sb = singles.tile([128, ES, H], f32)

            def bcast_load(src_ap):
                t_ = singles.tile([128, H], f32)
                nc.sync.dma_start(t_[:], src_ap.to_broadcast([128, H]))
                return t_

            c1_bc = bcast_load(c1.ap()) if flags["c1"] else None
            c2_bc = [bcast_load(c2s.ap()[t_i:t_i + 1, :]) if flags["c2"][t_i]
                     else None for t_i in range(T)]
            c3_bc = bcast_load(c3.ap()) if flags["c3"] else None
            gagg_bc = bcast_load(gagg.ap()) if flags["gagg"] else None
            bagg_bc = bcast_load(bagg.ap()) if flags["bagg"] else None

            # ---- whole pipeline, optionally repeated for benchmarking ----
            for _rep in range(repeat):
                _pipeline(nc, tile, mybir, AF, rg, flags if not nocc else
                          dict(flags, nocc=True), _rep,
                          pools=(singles, apool, hbpool, hshpool, work,
                                 pspool, psoutpool),
                          tens=(aT, aneT, out, ag_in, ag_out),
                          sbufs=(xct_sb, wi_sb, w2_sb, nft_sb, wn_sb, wm_sb,
                                 ones_bf, onescol, eps_sb, h0_sb),
                          bcs=(c1_bc, c2_bc, c3_bc, gagg_bc, bagg_bc))

    nc.compile()
    return nc


def _pipeline(nc, tile, mybir, AF, rg, flags, _rep, pools, tens, sbufs, bcs):
    import concourse.mybir as mybir  # noqa: F811
    f32 = mybir.dt.float32
    bf16 = mybir.dt.bfloat16
    f8 = mybir.dt.float8e4
    (singles, apool, hbpool, hshpool, work, pspool, psoutpool) = pools
    (aT, aneT, out, ag_in, ag_out) = tens
    (xct_sb, wi_sb, w2_sb, nft_sb, wn_sb, wm_sb,
     ones_bf, onescol, eps_sb, h0_sb) = sbufs
    (c1_bc, c2_bc, c3_bc, gagg_bc, bagg_bc) = bcs

    if True:
        if True:
            # ---- h0 = relu(X @ W_init + b_init), per-core edge shard ----
            hsh = hshpool.tile([128, ES, H], bf16, tag="hsh")
            for es in range(ES):
                ps = pspool.tile([128, H], f32, tag="ps")
                nc.tensor.matmul(ps[:], lhsT=xct_sb[:, es * 128:(es + 1) * 128],
                                 rhs=wi_sb[:], start=True, stop=True)
                if c1_bc is not None:
                    tmp = work.tile([128, H], f32, tag="tmp")
                    nc.vector.tensor_add(tmp[:], ps[:], c1_bc[:])
                    nc.scalar.activation(h0_sb[:, es], tmp[:], AF.Relu)
                else:
                    nc.scalar.activation(h0_sb[:, es], ps[:], AF.Relu)
                nc.vector.tensor_copy(hsh[:, es], h0_sb[:, es])

            def gather_h(t_idx, hsh_tile):
                nc.sync.dma_start(
                    ag_in[t_idx].ap().rearrange("(es p) h -> p es h", p=128),
                    hsh_tile[:])
                if flags.get("nocc"):
                    # timeline-sim variant: no collective support; emulate the
                    # gather's local DMA traffic by reading the shard 8x.
                    src = ag_in[t_idx].ap().rearrange("(k p) h -> p k h", p=128)
                    hb = []
                    for g in range(NCORES):
                        hg = hbpool.tile([128, KT // NCORES, H], bf16,
                                         tag=f"hb{g}", name=f"hb_{t_idx}_{g}")
                        nc.sync.dma_start(hg[:], src)
                        hb.append(hg)
                    return hb
                nc.gpsimd.collective_compute(
                    "AllGather", mybir.AluOpType.bypass, replica_groups=rg,
                    ins=[ag_in[t_idx].ap().opt()],
                    outs=[ag_out[t_idx].ap().opt()])
                src = ag_out[t_idx].ap().rearrange("(k p) h -> p k h", p=128)
                # one tile per 8-k chunk so the first matmuls can start as
                # soon as the first chunk lands instead of after all 4 MB
                hb = []
                for g in range(NCORES):
                    hg = hbpool.tile([128, KT // NCORES, H], bf16,
                                     tag=f"hb{g}", name=f"hb_{t_idx}_{g}")
                    nc.sync.dma_start(hg[:], src[:, g * 8:(g + 1) * 8])
                    hb.append(hg)
                return hb

            hb = gather_h(0, hsh)

            # ---- LN over the transposed message block, shared by both the
            # edge loop and the node-aggregation stage ----
            def ln_transposed(ps_m, width, n_et):
                """ps_m[half][et] psum tiles [128, 512] of m.T; returns a
                centered*rstd fp32 tile cln [128, 2, width]."""
                mT = work.tile([128, 2, width], bf16, tag=f"mT{width}")
                sq = work.tile([128, 2, width], bf16, tag=f"sq{width}")
                for half in range(2):
                    for et in range(n_et):
                        esl = slice(et * 512, (et + 1) * 512)
                        nc.vector.tensor_copy(mT[:, half, esl],
                                              ps_m[half][et][:])
                        # square straight from PSUM on the ACT engine:
                        # removes a serial DVE stage between the matmul tail
                        # and the stats matmuls
                        nc.scalar.activation(sq[:, half, esl],
                                             ps_m[half][et][:], AF.Square)
                cln = work.tile([128, 2, width], f32, tag=f"cln{width}")
                for et in range(n_et):
                    esl = slice(et * 512, (et + 1) * 512)
                    psmean = pspool.tile([128, 512], f32, tag="ps")
                    nc.tensor.matmul(psmean[:], lhsT=ones_bf[:],
                                     rhs=mT[:, 0, esl], start=True, stop=False)
                    nc.tensor.matmul(psmean[:], lhsT=ones_bf[:],
                                     rhs=mT[:, 1, esl], start=False, stop=True)
                    pssq = pspool.tile([128, 512], f32, tag="ps")
                    nc.tensor.matmul(pssq[:], lhsT=ones_bf[:],
                                     rhs=sq[:, 0, esl], start=True, stop=False)
                    nc.tensor.matmul(pssq[:], lhsT=ones_bf[:],
                                     rhs=sq[:, 1, esl], start=False, stop=True)
                    mean = work.tile([128, 512], f32, tag="mean")
                    nc.vector.tensor_scalar_mul(mean[:], psmean[:], 1.0 / H)
                    var = work.tile([128, 512], f32, tag="var")
                    nc.vector.tensor_scalar_mul(var[:], pssq[:], 1.0 / H)
                    msq = work.tile([128, 512], f32, tag="msq")
                    nc.vector.tensor_mul(msq[:], mean[:], mean[:])
                    nc.vector.tensor_sub(var[:], var[:], msq[:])
                    rstd = work.tile([128, 512], f32, tag="rstd")
                    nc.scalar.activation(rstd[:], var[:], AF.Sqrt,
                                         bias=eps_sb[:], scale=1.0)
                    nc.vector.reciprocal(out=rstd[:], in_=rstd[:])
                    for half in range(2):
                        nc.vector.tensor_sub(cln[:, half, esl],
                                             ps_m[half][et][:], mean[:])
                        nc.vector.tensor_mul(cln[:, half, esl],
                                             cln[:, half, esl], rstd[:])
                return cln

            # ---- T message-passing iterations ----
            for t in range(T):
                ps_m = [[pspool.tile([128, 512], f32, tag="ps",
                                     name=f"psm_{_rep}_{t}_{_h}_{_e}")
                         for _e in range(ET)] for _h in range(2)]
                for k in range(KT):
                    a_sb = apool.tile([128, EC], f8, tag="a")
                    nc.sync.dma_start(a_sb[:], aT[k * 128:(k + 1) * 128, :])
                    for half in range(2):
                        w = hb[k // 8][:, k % 8, half * 128:(half + 1) * 128]
                        for et in range(ET):
                            nc.tensor.matmul(
                                ps_m[half][et][:], lhsT=w,
                                rhs=a_sb[:, et * 512:(et + 1) * 512],
                                start=(k == 0), stop=(k == KT - 1))

                cln = ln_transposed(ps_m, EC, ET)

                hsh2 = hshpool.tile([128, ES, H], bf16, tag="hsh")
                for es in range(ES):
                    sl = slice(es * 128, (es + 1) * 128)
                    psu = pspool.tile([128, H], f32, tag="ps")
                    nc.tensor.matmul(psu[:], lhsT=cln[:, 0, sl],
                                     rhs=w2_sb[:, t, 0], start=True, stop=False)
                    nc.tensor.matmul(psu[:], lhsT=cln[:, 1, sl],
                                     rhs=w2_sb[:, t, 1], start=False, stop=True)
                    tmp = work.tile([128, H], f32, tag="tmp")
                    nc.vector.tensor_add(tmp[:], psu[:], h0_sb[:, es])
                    if c2_bc[t] is not None:
                        nc.vector.tensor_add(tmp[:], tmp[:], c2_bc[t][:])
                    nc.vector.tensor_scalar_max(hsh2[:, es], tmp[:], 0.0)

                hb = gather_h(t + 1, hsh2)

            # ---- node aggregation: m_v.T = h.T @ adj_ne_shard.T ----
            ps_mv = [[pspool.tile([128, 512], f32, tag="ps",
                                  name=f"psmv_{_rep}_{_h}")] for _h in range(2)]
            for k in range(KT):
                a_sb = apool.tile([128, NNC], f8, tag="ane")
                nc.sync.dma_start(a_sb[:], aneT[k * 128:(k + 1) * 128, :])
                for half in range(2):
                    nc.tensor.matmul(
                        ps_mv[half][0][:],
                        lhsT=hb[k // 8][:, k % 8, half * 128:(half + 1) * 128],
                        rhs=a_sb[:], start=(k == 0), stop=(k == KT - 1))

            cln_v = ln_transposed(ps_mv, NNC, 1)

            # ---- h_v = relu(nf @ Wagg[:64] + m_v_ln @ Wagg[64:] + c3);
            #      LN again; column-sum over nodes ----
            ps_out = psoutpool.tile([1, H], f32, tag="psout")
            for ns in range(NS):
                sl = slice(ns * 128, (ns + 1) * 128)
                ps_hv = pspool.tile([128, H], f32, tag="ps")
                nc.tensor.matmul(ps_hv[:], lhsT=nft_sb[:, sl], rhs=wn_sb[:],
                                 start=True, stop=False)
                nc.tensor.matmul(ps_hv[:], lhsT=cln_v[:, 0, sl],
                                 rhs=wm_sb[:, 0], start=False, stop=False)
                nc.tensor.matmul(ps_hv[:], lhsT=cln_v[:, 1, sl],
                                 rhs=wm_sb[:, 1], start=False, stop=True)
                hv = work.tile([128, H], f32, tag="hv")
                if c3_bc is not None:
                    nc.vector.tensor_add(hv[:], ps_hv[:], c3_bc[:])
                    nc.vector.tensor_scalar_max(hv[:], hv[:], 0.0)
                else:
                    nc.scalar.activation(hv[:], ps_hv[:], AF.Relu)
                stats = work.tile([128, 6], f32, tag="stats")
                nc.vector.bn_stats(out=stats[:], in_=hv[:])
                mv2 = work.tile([128, 2], f32, tag="mv2")
                nc.vector.bn_aggr(out=mv2[:], in_=stats[:])
                rstd2 = work.tile([128, 1], f32, tag="rstd2")
                nc.scalar.activation(rstd2[:], mv2[:, 1:2], AF.Sqrt,
                                     bias=eps_sb[:], scale=1.0)
                nc.vector.reciprocal(out=rstd2[:], in_=rstd2[:])
                ln2 = work.tile([128, H], f32, tag="ln2")
                nc.vector.tensor_scalar(
                    out=ln2[:], in0=hv[:], scalar1=mv2[:, 0:1],
                    scalar2=rstd2[:], op0=mybir.AluOpType.subtract,
                    op1=mybir.AluOpType.mult)
                if gagg_bc is not None:
                    nc.vector.tensor_mul(ln2[:], ln2[:], gagg_bc[:])
                if bagg_bc is not None:
                    nc.vector.tensor_add(ln2[:], ln2[:], bagg_bc[:])
                nc.tensor.matmul(ps_out[:], lhsT=onescol[:], rhs=ln2[:],
                                 start=(ns == 0), stop=(ns == NS - 1))

            out_sb = work.tile([1, H], f32, tag="osb")
            nc.vector.tensor_copy(out_sb[:], ps_out[:])
            nc.sync.dma_start(out[:, :], out_sb[:])


def prepare(inputs):
    """Host-side prep: returns (nc, in_maps) for run_bass_kernel_spmd."""
    f = {k: np.ascontiguousarray(np.asarray(v), dtype=np.float32)
         for k, v in inputs.items()}

    X = np.concatenate(
        [f["edge_aligned_node_features"], f["dir_edge_features"]], axis=1)

    # Fold LN gains into the downstream weights (host-side, exact fp32).
    g_p, b_p = f["ln_pass_g"], f["ln_pass_b"]
    g_a, b_a = f["ln_agg_g"], f["ln_agg_b"]
    W2s = (g_p[:, None] * f["W_pass"]).astype(np.float32)          # [T,H,H]
    c2s = (b_p @ f["W_pass"] + f["b_pass"]).astype(np.float32)     # [T,H]
    Wm = (g_a[:, None] * f["W_agg"][NODE_D:]).astype(np.float32)   # [H,H]
    c3 = (b_a @ f["W_agg"][NODE_D:] + f["b_agg"]).astype(np.float32)

    Wi = np.zeros((128, H), np.float32)
    Wi[:NODE_D + EDGE_D] = f["W_init"]
    Wn = np.zeros((128, H), np.float32)
    Wn[:NODE_D] = f["W_agg"][:NODE_D]

    flags = {
        "c1": bool(np.any(f["b_init"])),
        "c2": [bool(np.any(c2s[t])) for t in range(T)],
        "c3": bool(np.any(c3)),
        "gagg": not np.all(g_a == 1.0),
        "bagg": bool(np.any(b_a)),
    }
    key = (flags["c1"], tuple(flags["c2"]), flags["c3"], flags["gagg"],
           flags["bagg"])
    if key not in _prog_cache:
        _prog_cache[key] = _build(flags)
    nc = _prog_cache[key]

    shared = {
        "Wi": Wi,
        "W2s": W2s,
        "Wn": Wn,
        "Wm": Wm,
        "c1": f["b_init"].reshape(1, H),
        "c2s": c2s,
        "c3": c3.reshape(1, H),
        "gagg": g_a.reshape(1, H),
        "bagg": b_a.reshape(1, H),
    }
    in_maps = []
    for c in range(NCORES):
        er = slice(c * EC, (c + 1) * EC)
        nr = slice(c * NNC, (c + 1) * NNC)
        XcT = np.zeros((128, EC), np.float32)
        XcT[:NODE_D + EDGE_D] = X[er].T
        nfT = np.zeros((128, NNC), np.float32)
        nfT[:NODE_D] = f["node_features"][nr].T
        in_maps.append(dict(
            shared,
            aT=np.ascontiguousarray(f["adj_ee"][er].T).astype(F8),
            aneT=np.ascontiguousarray(f["adj_ne"][nr].T).astype(F8),
            XcT=XcT,
            nfT=nfT,
        ))
    return nc, in_maps


def kernel(**inputs) -> np.ndarray:
    global LAST_RESULT
    from concourse.bass_utils import run_bass_kernel_spmd

    nc, in_maps = prepare(inputs)
    LAST_RESULT = run_bass_kernel_spmd(nc, in_maps, list(range(NCORES)))
    parts = [LAST_RESULT.results[c]["out"] for c in range(NCORES)]
    return np.sum(parts, axis=0, dtype=np.float32).reshape(1, H)
